# revision 1
# baseline (speedup 1.0000x reference)
"""Bilinear-model-topk kernel for 8 TRN2 NeuronCores.

Model (reference):
    x  = rms(x)                 # rms(v) = v / sqrt(sum(v^2))  (no 1/d)
    x  = x[:, idx]              # gather -> [b, d]
    y1 = rms(bilinear(x, B1))   # bilinear(x, B)[b,s] = x^T B_s x
    y2 = bilinear(y1, B2)
    out = y2 @ W_out.T + bias

Scale-invariance reduction (exact math): rms() divides by a positive
per-row scalar, and bilinear is quadratic, so
    rms(bilinear(c * x, B)) == rms(bilinear(x, B))        for c > 0
Hence the first rms() can be dropped entirely, and the second rms()
collapses to scaling y2 by 1/sum(y1_raw^2) per row:
    y2 = bilinear(rms(y1_raw), B2) = bilinear(y1_raw, B2) / sum(y1_raw^2)
The device therefore computes:
    y1_raw = bilinear(x_gathered_raw, B1)      (s sharded 64/core)
    AllGather y1_raw across the 8 cores
    rr = 1 / sum_s(y1_raw^2)  per row
    y2s = bilinear(y1_raw, B2) * rr            (s sharded 64/core)
    out_partial = y2s_shard @ W_out[:, shard].T
Host sums the 8 partials and adds bias.

Per-core inner loop (per output neuron s):
    DMA B_s [512, 512] (1 MB, contiguous) as SBUF [128, 4*512]
    4 accumulating matmuls: psum[64,512] += xT_k[128,64].T @ B_s[k][128,512]
    tensor_tensor_reduce: y[b, s] = sum_e psum[b,e] * x[b,e]

Built on bacc.Bacc (not plain Bass): walrus allows at most one sync
wait per hardware instruction, and Bacc's compile() legalizes the
Tile-emitted waits to satisfy that.
"""

import os

import numpy as np

N_CORES = 8
B = 64          # batch
D = 512         # bilinear width (s and contraction dims)
D_FULL = 1024   # pre-gather width
OUT = 1024      # output width
S_SH = D // N_CORES   # 64 output neurons per core
KC = D // 128         # 4 k-chunks of 128

# "f32": exact fp32 matmul (4 cyc/row). "f32r": fp32 storage, FP22
# multiply (1 cyc/row). "bf16": bf16 B storage (half HBM traffic).
MODE = os.environ.get("BILINEAR_KERNEL_MODE", "f32r")

_NC_CACHE = {}


def _build(mode, s_sh=S_SH, use_cc=True):
    from concourse import bacc, bass, masks, mybir  # noqa: F401
    from concourse.tile import TileContext

    f32 = mybir.dt.float32
    if mode == "bf16":
        b_store = mybir.dt.bfloat16    # B storage dtype (DRAM + SBUF)
        x_store = mybir.dt.bfloat16    # stationary-operand SBUF dtype
    elif mode == "f32r":
        # walrus requires fp32r-matmul inputs to be *typed* f32r at the
        # producer, so DRAM + SBUF tiles are declared f32r.
        b_store = mybir.dt.float32r
        x_store = mybir.dt.float32r
    else:
        b_store = f32
        x_store = f32

    # Bacc (not plain Bass): its compile() pass legalizes sync waits —
    # splits >1-wait instructions and moves matmul waits onto ldweights —
    # without which walrus codegen fails ("Too many sync wait commands").
    nc = bacc.Bacc(None, target_bir_lowering=False, num_devices=N_CORES)

    xg_d = nc.dram_tensor("xg", [B, D], f32, kind="ExternalInput")
    xgT_d = nc.dram_tensor("xgT", [D, B], x_store, kind="ExternalInput")
    b1_d = nc.dram_tensor("b1s", [s_sh, D, D], b_store, kind="ExternalInput")
    b2_d = nc.dram_tensor("b2s", [s_sh, D, D], b_store, kind="ExternalInput")
    wo_d = nc.dram_tensor("woT", [s_sh, OUT], f32, kind="ExternalInput")
    out_d = nc.dram_tensor("out", [B, OUT], f32, kind="ExternalOutput")
    # Collective bounce buffers (internal DRAM; output must be Shared).
    # Two half-gathers: the first launches mid-layer-1 (fully hidden) and
    # keeps ncfw warm so the second's exposed latency is smaller.
    sh2 = s_sh // 2
    y1loc_a = nc.dram_tensor("y1loc_a", [B, sh2], f32)
    y1loc_b = nc.dram_tensor("y1loc_b", [B, sh2], f32)
    y1full_a = nc.dram_tensor("y1full_a", [N_CORES, B, sh2], f32, addr_space="Shared")
    y1full_b = nc.dram_tensor("y1full_b", [N_CORES, B, sh2], f32, addr_space="Shared")

    with TileContext(nc) as tc:
        with (
            tc.tile_pool(name="constp", bufs=1) as constp,
            tc.tile_pool(name="xp", bufs=1) as xp,
            tc.tile_pool(name="bp", bufs=11) as bp,
            tc.tile_pool(name="pps", bufs=6, space="PSUM") as pps,
            tc.tile_pool(name="ppt", bufs=1, space="PSUM") as ppt,
            tc.tile_pool(name="ppo", bufs=1, space="PSUM") as ppo,
        ):
            ident = constp.tile([128, 128], f32, tag="ident")
            masks.make_identity(nc, ident[:])

            # Small loads go on gpsimd (SWDGE) so the sync (HWDGE) FIFO
            # carries nothing but the B stream.
            xg_sb = xp.tile([B, D], f32, tag="xg")
            nc.gpsimd.dma_start(xg_sb[:], xg_d[:])
            wo_sb = xp.tile([s_sh, OUT], f32, tag="wo")
            nc.gpsimd.dma_start(wo_sb[:], wo_d[:])
            xT_sb = xp.tile([128, KC * B], x_store, tag="xT")
            nc.gpsimd.dma_start(
                xT_sb[:].rearrange("p (k b) -> p k b", k=KC),
                xgT_d[:].rearrange("(k p) b -> p k b", p=128),
            )

            junk_sb = xp.tile([B, 512], f32, tag="junk")

            def bilinear(b_dram, lhs_sb, xvec_sb, y_sb, half_hook=None):
                # 2 MB DMAs (an s-pair per transfer) amortize per-DMA
                # overhead; deep bp pool bridges the collective latency.
                # d0-interleaved contraction: partition p of chunk k holds
                # d0 = 4p + k, so each partition's DMA run is 8 KB
                # contiguous (4 consecutive B rows). The contraction order
                # is free as long as lhs rows match (host permutes xgT; the
                # y1 transposes read stride-4 column slices).
                for pr in range(s_sh // 2):
                    bt = bp.tile([128, 2 * KC * 512], b_store, tag="bt")
                    nc.sync.dma_start(
                        bt[:].rearrange("p (s2 r n) -> p s2 r n", s2=2, r=KC),
                        b_dram[2 * pr:2 * pr + 2].rearrange(
                            "s2 (p r) n -> p s2 r n", p=128
                        ),
                    )
                    for s2 in range(2):
                        ps = pps.tile([B, 512], f32, tag="ps")
                        for k in range(KC):
                            nc.tensor.matmul(
                                ps[:],
                                lhs_sb[:, k * B:(k + 1) * B],
                                bt[:, (s2 * KC + k) * 512:(s2 * KC + k + 1) * 512],
                                start=(k == 0),
                                stop=(k == KC - 1),
                            )
                        # second contraction: DVE multiplies in place in
                        # PSUM, idle ScalarE reduces straight from PSUM via
                        # activation accumulate (fast PSUM read; also avoids
                        # an SBUF scratch whose slot recycle paced the
                        # pipeline). tensor_tensor_reduce itself dies with
                        # an INTERNAL error on this runtime.
                        s = 2 * pr + s2
                        nc.vector.tensor_mul(ps[:], ps[:], xvec_sb[:])
                        nc.scalar.activation(
                            junk_sb[:], ps[:],
                            mybir.ActivationFunctionType.Copy,
                            accum_out=y_sb[:, s:s + 1],
                        )
                    if half_hook is not None and pr == s_sh // 4 - 1:
                        half_hook()

            # ---- layer 1 (raw, unnormalized input) ----
            y1_sb = xp.tile([B, s_sh], f32, tag="y1")
            y1f_sb = xp.tile([B, D], f32, tag="y1f")
            y1f_3d = y1f_sb[:].rearrange("b (c j) -> b c j", c=N_CORES)

            def gather_half(loc, full, lo, hi):
                nc.scalar.dma_start(loc[:], y1_sb[:, lo:hi])
                if use_cc:
                    nc.gpsimd.collective_compute(
                        "AllGather",
                        mybir.AluOpType.bypass,
                        replica_groups=[list(range(N_CORES))],
                        ins=[loc[:]],
                        outs=[full[:]],
                    )
                else:
                    for c in range(N_CORES):
                        nc.gpsimd.dma_start(full[c], loc[:])
                # gpsimd (not scalar): keeps the wait-on-collective out of
                # the ACT FIFO, which still has layer-1 accums to run
                nc.gpsimd.dma_start(
                    y1f_3d[:, :, lo:hi],
                    full[:].rearrange("c b j -> b c j"),
                )

            bilinear(
                b1_d, xT_sb, xg_sb, y1_sb,
                half_hook=lambda: gather_half(y1loc_a, y1full_a, 0, sh2),
            )
            gather_half(y1loc_b, y1full_b, sh2, s_sh)

            # rr = 1 / sum_s y1_raw^2  (both rms scales collapse into this)
            ss_sb = xp.tile([B, 1], f32, tag="ss")
            nc.scalar.activation(
                junk_sb[:], y1f_sb[:], mybir.ActivationFunctionType.Square,
                accum_out=ss_sb[:],
            )
            rr_sb = xp.tile([B, 1], f32, tag="rr")
            nc.vector.reciprocal(rr_sb[:], ss_sb[:])

            # transpose y1 for the layer-2 stationary operand; chunk k is
            # the stride-4 column slice y1f[:, k::4] to match the
            # d0-interleaved B layout (d0 = 4p + k on partition p)
            y1T_sb = xp.tile([128, KC * B], x_store, tag="y1T")
            y1f_il = y1f_sb[:].rearrange("b (e r) -> b r e", r=KC)
            for k in range(KC):
                tp = ppt.tile([128, B], f32, tag="tp")
                nc.tensor.transpose(tp[:], y1f_il[:, k, :], ident[:B, :B])
                nc.vector.tensor_copy(y1T_sb[:, k * B:(k + 1) * B], tp[:])

            # ---- layer 2 (raw y1) ----
            y2_sb = xp.tile([B, s_sh], f32, tag="y2")
            bilinear(b2_d, y1T_sb, y1f_sb, y2_sb)

            # scale by rr (this is the layer-1 rms applied through the
            # quadratic), transpose, and contract with W_out slice
            y2s_sb = xp.tile([B, s_sh], f32, tag="y2s")
            nc.vector.tensor_scalar_mul(y2s_sb[:], y2_sb[:], rr_sb[:, 0:1])
            tp2 = ppt.tile([128, B], f32, tag="tp")
            nc.tensor.transpose(tp2[:B, :], y2s_sb[:], ident[:B, :B])
            y2T_sb = xp.tile([B, B], f32, tag="y2T")
            nc.vector.tensor_copy(y2T_sb[:], tp2[:B, :])

            out_sb = xp.tile([B, OUT], f32, tag="outsb")
            for j in range(OUT // 512):
                po = ppo.tile([B, 512], f32, tag="po")
                nc.tensor.matmul(
                    po[:],
                    y2T_sb[:],
                    wo_sb[:, j * 512:(j + 1) * 512],
                    start=True,
                    stop=True,
                )
                nc.vector.tensor_copy(out_sb[:, j * 512:(j + 1) * 512], po[:])
            nc.scalar.dma_start(out_d[:], out_sb[:])

    nc.compile()
    return nc


def _get_nc(mode):
    use_cc = os.environ.get("BILINEAR_NO_CC", "0") != "1"
    key = (mode, use_cc)
    if key not in _NC_CACHE:
        _NC_CACHE[key] = _build(mode, use_cc=use_cc)
    return _NC_CACHE[key]


def _make_in_maps(x, B1, B2, W_out, input_idxs, mode):
    idx = np.asarray(input_idxs).astype(np.int64)
    x = np.asarray(x, dtype=np.float32)
    xg = np.ascontiguousarray(x[:, idx])  # [64, 512] raw gathered input
    # d0-interleaved stationary layout: row k*128+p holds x[:, 4p+k]
    xgT = np.ascontiguousarray(
        xg.T.reshape(128, KC, B).transpose(1, 0, 2).reshape(D, B)
    )
    woT = np.ascontiguousarray(np.asarray(W_out, dtype=np.float32).T)  # [512,1024]
    B1 = np.asarray(B1, dtype=np.float32)
    B2 = np.asarray(B2, dtype=np.float32)
    if mode == "bf16":
        import ml_dtypes
        B1 = B1.astype(ml_dtypes.bfloat16)
        B2 = B2.astype(ml_dtypes.bfloat16)
        xgT = xgT.astype(ml_dtypes.bfloat16)
    in_maps = []
    for c in range(N_CORES):
        sl = slice(c * S_SH, (c + 1) * S_SH)
        in_maps.append({
            "xg": xg,
            "xgT": xgT,
            "b1s": np.ascontiguousarray(B1[sl]),
            "b2s": np.ascontiguousarray(B2[sl]),
            "woT": np.ascontiguousarray(woT[sl]),
        })
    return in_maps


def run_with_results(x, B1, B2, W_out, bias_out, input_idxs, mode=None, **spmd_kwargs):
    """Run the distributed kernel; returns (output, BassKernelResults)."""
    from concourse.bass_utils import run_bass_kernel_spmd

    mode = mode or MODE
    nc = _get_nc(mode)
    in_maps = _make_in_maps(x, B1, B2, W_out, input_idxs, mode)
    res = run_bass_kernel_spmd(
        nc, in_maps, core_ids=list(range(N_CORES)), **spmd_kwargs
    )
    acc = np.zeros((B, OUT), dtype=np.float32)
    for r in res.results:
        acc += r["out"]
    out = acc + np.asarray(bias_out, dtype=np.float32)[None, :]
    return out.astype(np.float32), res


def kernel(x, B1, B2, W_out, bias_out, input_idxs):
    out, _ = run_with_results(x, B1, B2, W_out, bias_out, input_idxs)
    return out

